# revision 10
# baseline (speedup 1.0000x reference)
"""GAT message-passing kernel for Trainium2, 8 NeuronCores — v2.

Problem (hardcoded): B=4, N=1024, H=F=O=G=128, E=16.
  features = concat([n_features, hidden], -1)            [B,N,256]
  values   = features @ W_m + b_m                        [B,N,128]
  logits   = att1 + att2^T + (e_features@w_ae) + att_g   [B,N,N]
  coefs    = softmax(leaky_relu(logits) + (adj-1)*1e9)
  out      = coefs @ values + features @ W_skip + b_skip

Sharding: 8 cores = (batch b = core//2) x (row half = core%2).
Each core handles 512 query rows of one batch. No collectives.

v2 design (from trace analysis of v1 @156us; DMA stream floor ~101us):
  - DVE instruction stream contains ONLY the steady-state work: per
    512-key half  mul(2x) -> tree L1/L2(2x) -> L3 into contiguous pair
    buffer -> STT final (no stride-16 singles)  ~9.6us < 11.9us DMA
    period.  v1 spent 13.3us/half (strided tree tail) and its DVE
    stream was blocked 33us behind phase-0.
  - att1+att_g+biases ride the ScalarE activation bias (per-partition);
    att2 is broadcast across partitions once via PE and added by
    GpSimd per half, so logit assembly costs DVE nothing.
  - Prelu (parametric relu, alpha=.01) shares the activation table
    with Exp/Copy -> zero ACT_TABLE_LOADs (v1 burned 21.8us on 17).
  - softmax denominator via TTR accum chaining per chunk; 1/s via
    vector.reciprocal (kills the Ln/Exp table thrash).
  - software-pipelined emission: acts lag one half, A@V lags two, ef
    DMA triggers always one period ahead on the gpsimd queue.
  - first and last halves stream in 256-key quarters to cut the
    pipeline head (first mul ~11.5us) and tail (~9us after last byte).
"""

import os
import numpy as np

B, N, H, F, E, G, O = 4, 1024, 128, 128, 16, 128, 128
DIN = F + H
NCORES = 8
ROWS = N // 2          # query rows per core
RT = ROWS // 128       # row tiles per core
KC = N // 128          # key chunks per row
KH = 2                 # key halves per row
KHW = N // KH          # keys per half

_cache = {}


def _build(stage=4):
    from contextlib import ExitStack
    import concourse.bacc as bacc
    import concourse.tile as tile
    import concourse.mybir as mybir
    import concourse.bass as bass

    fp32 = mybir.dt.float32
    bf16 = mybir.dt.bfloat16
    ALU = mybir.AluOpType
    AF = mybir.ActivationFunctionType
    relu_mode = bool(os.environ.get("GAT_LRELU_MODE"))  # Relu+STT fallback
    use_gps = os.environ.get("GAT_GPS", "0") == "1"    # gpsimd compute ops
    use_ttr = os.environ.get("GAT_TTR", "0") == "1"    # tensor_tensor_reduce
    use_recip = os.environ.get("GAT_RECIP", "1") == "1"  # vector.reciprocal
    use_p4 = os.environ.get("GAT_P4", "0") == "1"      # P=4 bias matmul

    nc = bacc.Bacc("TRN2", target_bir_lowering=False, debug=False,
                   num_devices=NCORES)

    # ---- per-core I/O -------------------------------------------------
    ef_in = nc.dram_tensor("ef", [ROWS, N, E], fp32, kind="ExternalInput")
    adj_in = nc.dram_tensor("adj", [ROWS, N], fp32, kind="ExternalInput")
    nfk_in = nc.dram_tensor("nfk", [N, F], fp32, kind="ExternalInput")
    hidk_in = nc.dram_tensor("hidk", [N, H], fp32, kind="ExternalInput")
    g_in = nc.dram_tensor("g", [G, 1], fp32, kind="ExternalInput")
    Wm_in = nc.dram_tensor("Wm", [DIN, O], fp32, kind="ExternalInput")
    bm_in = nc.dram_tensor("bm", [1, O], fp32, kind="ExternalInput")
    Wsk_in = nc.dram_tensor("Wsk", [DIN, O], fp32, kind="ExternalInput")
    bsk_in = nc.dram_tensor("bsk", [1, O], fp32, kind="ExternalInput")
    wa1_in = nc.dram_tensor("wa1", [DIN, 1], fp32, kind="ExternalInput")
    wa2_in = nc.dram_tensor("wa2", [DIN, 1], fp32, kind="ExternalInput")
    wae_in = nc.dram_tensor("wae", [1, E], fp32, kind="ExternalInput")
    wag_in = nc.dram_tensor("wag", [G, 1], fp32, kind="ExternalInput")
    bs4_in = nc.dram_tensor("bs4", [4, 1], fp32, kind="ExternalInput")
    bs14_in = nc.dram_tensor("bs14", [1, 4], fp32, kind="ExternalInput")
    ident_in = nc.dram_tensor("ident", [128, 128], fp32, kind="ExternalInput")
    out_t = nc.dram_tensor("out", [ROWS, O], fp32, kind="ExternalOutput")

    with tile.TileContext(nc) as tc:
        with ExitStack() as ctx:
            singles = ctx.enter_context(tc.tile_pool(name="singles", bufs=1))
            efp = ctx.enter_context(tc.tile_pool(name="efp", bufs=4))
            wefp = ctx.enter_context(tc.tile_pool(name="wefp", bufs=2))
            pairp = ctx.enter_context(tc.tile_pool(name="pairp", bufs=2))
            work = ctx.enter_context(tc.tile_pool(name="work", bufs=2))
            adjp = ctx.enter_context(tc.tile_pool(name="adjp", bufs=3))
            small = ctx.enter_context(tc.tile_pool(name="small", bufs=2))
            psT = ctx.enter_context(tc.tile_pool(name="psT", bufs=3, space="PSUM"))
            psR = ctx.enter_context(tc.tile_pool(name="psR", bufs=2, space="PSUM"))
            psS = ctx.enter_context(tc.tile_pool(name="psS", bufs=2, space="PSUM"))

            # ============ GpSimd head: memsets + wae bcast ==============
            eng0 = nc.gpsimd if use_gps else nc.vector
            ones_bf = singles.tile([1, 512], bf16)
            eng0.memset(ones_bf, 1.0)
            ones128b = ones_bf[:, :128]
            ones_f = singles.tile([1, 128], fp32)
            eng0.memset(ones_f, 1.0)
            ones4 = singles.tile([4, 1], fp32)
            eng0.memset(ones4, 1.0)
            w_tile = singles.tile([128, E], bf16)       # w_ae bcast to parts
            nc.gpsimd.dma_start(out=w_tile, in_=bass.AP(
                tensor=wae_in, offset=0, ap=[[0, 128], [1, E]]))

            # ============ sync-queue loads (Q1) =========================
            ident_sb = singles.tile([128, 128], fp32)
            nc.sync.dma_start(out=ident_sb, in_=ident_in.ap())
            Wsk_sb = singles.tile([128, 2, O], fp32)
            nc.sync.dma_start(out=Wsk_sb, in_=Wsk_in.ap().rearrange(
                "(c p) o -> p c o", p=128))
            wa1_sb = singles.tile([128, 2, 1], fp32)
            nc.sync.dma_start(out=wa1_sb, in_=wa1_in.ap().rearrange(
                "(c p) o -> p c o", p=128))
            bsk_sb = singles.tile([1, O], fp32)
            nc.sync.dma_start(out=bsk_sb, in_=bsk_in.ap())
            g_sb = singles.tile([128, 1], fp32)
            nc.sync.dma_start(out=g_sb, in_=g_in.ap())
            wag_sb = singles.tile([128, 1], fp32)
            nc.sync.dma_start(out=wag_sb, in_=wag_in.ap())
            bs4_sb = singles.tile([4, 1], fp32)
            nc.sync.dma_start(out=bs4_sb, in_=bs4_in.ap())
            bs14_sb = singles.tile([1, 4], fp32)
            nc.sync.dma_start(out=bs14_sb, in_=bs14_in.ap())

            # persistent outputs of phase 0
            fTk0 = singles.tile([128, N], bf16)    # n_features^T (keys)
            fTk1 = singles.tile([128, N], bf16)    # hidden^T (keys)
            fTr0 = singles.tile([128, ROWS], fp32)  # fp32 copies for rows
            fTr1 = singles.tile([128, ROWS], fp32)
            V = singles.tile([128, KC, O], bf16)
            Wm_sb = singles.tile([128, 2, O], bf16)
            wa2_sb = singles.tile([128, 2, 1], bf16)
            bm_sb = singles.tile([1, O], bf16)
            att2b = singles.tile([128, N], fp32)   # att2 bcast to all parts
            att1p = singles.tile([128, RT], fp32)  # att1 + att_g + biases
            sc_sb = singles.tile([1, 1], fp32)

            with tc.tile_pool(name="ph0", bufs=1) as ph0:
                nfk_sb = ph0.tile([128, KC, F], fp32)
                nc.sync.dma_start(out=nfk_sb, in_=nfk_in.ap().rearrange(
                    "(c p) f -> p c f", p=128))
                hidk_sb = ph0.tile([128, KC, H], fp32)
                nc.sync.dma_start(out=hidk_sb, in_=hidk_in.ap().rearrange(
                    "(c p) f -> p c f", p=128))
                Wm_f = ph0.tile([128, 2, O], fp32)
                nc.sync.dma_start(out=Wm_f, in_=Wm_in.ap().rearrange(
                    "(c p) o -> p c o", p=128))
                wa2_f = ph0.tile([128, 2, 1], fp32)
                nc.sync.dma_start(out=wa2_f, in_=wa2_in.ap().rearrange(
                    "(c p) o -> p c o", p=128))
                bm_f = ph0.tile([1, O], fp32)
                nc.sync.dma_start(out=bm_f, in_=bm_in.ap())

                # sc = g@w_ag + (b_a1+b_a2+b_ae+b_ag)   [1,1]
                scps = psR.tile([1, 1], fp32, tag="ret")
                nc.tensor.matmul(scps, g_sb, wag_sb, start=True,
                                 stop=not use_p4)
                if use_p4:
                    nc.tensor.matmul(scps, bs4_sb, ones4,
                                     start=False, stop=True)
                    nc.scalar.copy(out=sc_sb, in_=scps)
                else:
                    nc.scalar.copy(out=sc_sb, in_=scps)
                    for i in range(4):
                        nc.vector.tensor_scalar_add(sc_sb, sc_sb,
                                                    bs14_sb[:, i:i + 1])

                # weight casts fp32 -> bf16 on ScalarE
                nc.scalar.copy(out=Wm_sb, in_=Wm_f)
                nc.scalar.copy(out=wa2_sb, in_=wa2_f)
                nc.scalar.copy(out=bm_sb, in_=bm_f)

                # features^T: per key chunk, bf16 everywhere + fp32 for
                # our own rows (rows of this core = keys[h*512:(h+1)*512],
                # h encoded host-side by which 512 rows of nfk/hidk are
                # ALSO this core's query rows; host passes row0 via rbase)
                # -> we transpose all 8 chunks; fp32 copies only for the
                # 4 chunks covering our rows, selected host-side by
                # passing nfk/hidk with our rows first?  Simpler: host
                # tells us nothing; fTr is filled from chunks rb..rb+3
                # with rb fixed at build time?  rb differs per core!
                # -> keep it data-independent: copy fp32 for ALL 8 chunks
                #    into fTrK [128, N], slice per row tile at use site.
                for kc in range(KC):
                    ksl = slice(kc * 128, (kc + 1) * 128)
                    tp = psT.tile([128, 128], fp32, tag="tp")
                    nc.tensor.transpose(tp, nfk_sb[:, kc, :], ident_sb)
                    nc.scalar.copy(out=fTk0[:, ksl], in_=tp)
                    tp2 = psT.tile([128, 128], fp32, tag="tp")
                    nc.tensor.transpose(tp2, hidk_sb[:, kc, :], ident_sb)
                    nc.scalar.copy(out=fTk1[:, ksl], in_=tp2)

                # values per key chunk (+b_m via ones-matmul)
                for kc in range(KC):
                    vps = psR.tile([128, O], fp32, tag="ret")
                    ksl = slice(kc * 128, (kc + 1) * 128)
                    nc.tensor.matmul(vps, fTk0[:, ksl], Wm_sb[:, 0, :],
                                     start=True, stop=False)
                    nc.tensor.matmul(vps, fTk1[:, ksl], Wm_sb[:, 1, :],
                                     start=False, stop=False)
                    nc.tensor.matmul(vps, ones128b, bm_sb,
                                     start=False, stop=True)
                    nc.scalar.copy(out=V[:, kc, :], in_=vps)

                # att2 over all keys, broadcast to 128 partitions
                att2_sb = ph0.tile([1, N], fp32)
                for khf in range(2):
                    ksl = slice(khf * 512, (khf + 1) * 512)
                    a2ps = psR.tile([1, 512], fp32, tag="ret")
                    nc.tensor.matmul(a2ps, wa2_sb[:, 0, :], fTk0[:, ksl],
                                     start=True, stop=False)
                    nc.tensor.matmul(a2ps, wa2_sb[:, 1, :], fTk1[:, ksl],
                                     start=False, stop=True)
                    nc.scalar.copy(out=att2_sb[:, ksl], in_=a2ps)
                for khf in range(2):
                    ksl = slice(khf * 512, (khf + 1) * 512)
                    bcps = psR.tile([128, 512], fp32, tag="ret")
                    nc.tensor.matmul(bcps, ones_f, att2_sb[:, ksl],
                                     start=True, stop=True)
                    nc.scalar.copy(out=att2b[:, ksl], in_=bcps)
            # ph0 pool closed; fp32 row-transposes below live in singles.

            # fTr (fp32 features^T for our rows): rows r of this core are
            # keys again, but the HOST passes nfr/hidr implicitly: our
            # rows are exactly nfk/hidk rows [h*512,(h+1)*512) and h is a
            # per-core runtime property.  Host-side we simply pass the
            # row block first via separate nfr tensors?  No: we instead
            # re-transpose from nfk_sb... which is freed.  Use fTk (bf16)
            # for att1/skip would lose precision.  Instead host passes
            # rbase as a 0/1 flag via... KISS: transpose from fTk? wrong
            # direction.  Host passes nfr/hidr slices like v1 (tiny).
            # (Handled below, outside ph0, via their own DMAs.)
            pass

            nfr_in = nc.dram_tensor("nfr", [ROWS, F], fp32,
                                    kind="ExternalInput")
            hidr_in = nc.dram_tensor("hidr", [ROWS, H], fp32,
                                     kind="ExternalInput")
            with tc.tile_pool(name="ph0b", bufs=1) as ph0b:
                nfr_sb = ph0b.tile([128, RT, F], fp32)
                nc.sync.dma_start(out=nfr_sb, in_=nfr_in.ap().rearrange(
                    "(c p) f -> p c f", p=128))
                hidr_sb = ph0b.tile([128, RT, H], fp32)
                nc.sync.dma_start(out=hidr_sb, in_=hidr_in.ap().rearrange(
                    "(c p) f -> p c f", p=128))
                for rc in range(RT):
                    rsl = slice(rc * 128, (rc + 1) * 128)
                    tp = psT.tile([128, 128], fp32, tag="tp")
                    nc.tensor.transpose(tp, nfr_sb[:, rc, :], ident_sb)
                    nc.scalar.copy(out=fTr0[:, rsl], in_=tp)
                    tp2 = psT.tile([128, 128], fp32, tag="tp")
                    nc.tensor.transpose(tp2, hidr_sb[:, rc, :], ident_sb)
                    nc.scalar.copy(out=fTr1[:, rsl], in_=tp2)

                # att1p[:, rt] = fTr(rt)@w_a1 + sc  (sc via rank-1 matmul)
                for rc in range(RT):
                    rsl = slice(rc * 128, (rc + 1) * 128)
                    aps = psR.tile([128, 1], fp32, tag="ret")
                    nc.tensor.matmul(aps, fTr0[:, rsl], wa1_sb[:, 0, :],
                                     start=True, stop=False)
                    nc.tensor.matmul(aps, fTr1[:, rsl], wa1_sb[:, 1, :],
                                     start=False, stop=False)
                    nc.tensor.matmul(aps, ones_f, sc_sb,
                                     start=False, stop=True)
                    nc.scalar.copy(out=att1p[:, rc:rc + 1], in_=aps)

            # ============ streaming pipeline ============================
            # halves: j = rt*2 + kh; first and last halves stream as two
            # 256-key quarters, the rest as one 512-key chunk.
            halves = []
            for rt in range(RT):
                for kh in range(2):
                    j = rt * 2 + kh
                    quartered = (j == 0) or (j == RT * 2 - 1)
                    chunks = [(0, 256), (256, 256)] if quartered \
                        else [(0, 512)]
                    halves.append(dict(rt=rt, kh=kh, chunks=chunks,
                                       last=(kh == 1)))
            J = len(halves)

            wta = w_tile[:]

            rts = [dict() for _ in range(RT)]   # per-row-tile state

            def ensure_rt(rt):
                st = rts[rt]
                if "acc" in st:
                    return st
                st["acc"] = work.tile([128, N], fp32, tag="acc", name=f"acc{rt}")
                st["coefs"] = work.tile([128, N], fp32, tag="coefs", name=f"coefs{rt}")
                st["s"] = []
                st["nseg"] = 0
                return st

            def emit_adj(rt):
                st = ensure_rt(rt)
                st["adj"] = adjp.tile([128, N], fp32, tag="adj", name=f"adj{rt}")
                rsl = slice(rt * 128, (rt + 1) * 128)
                nc.sync.dma_start(out=st["adj"], in_=adj_in[rsl, :])

            def emit_trig(j):
                hj = halves[j]
                rt, kh = hj["rt"], hj["kh"]
                rsl = slice(rt * 128, (rt + 1) * 128)
                hj["ef"] = efp.tile([128, KHW, E], bf16, tag="ef", name=f"ef{j}")
                for (c0, nk) in hj["chunks"]:
                    gk = slice(kh * KHW + c0, kh * KHW + c0 + nk)
                    nc.gpsimd.dma_start(out=hj["ef"][:, c0:c0 + nk, :],
                                        in_=ef_in[rsl, gk, :])

            def emit_dve_a(j):
                hj = halves[j]
                rt, kh = hj["rt"], hj["kh"]
                st = ensure_rt(rt)
                hj["wef"] = wefp.tile([128, KHW, E], bf16, tag="wef", name=f"wef{j}")
                hj["P"] = pairp.tile([128, KHW, 2], bf16, tag="P", name=f"P{j}")
                ef_t, wef, P = hj["ef"], hj["wef"], hj["P"]
                for (c0, nk) in hj["chunks"]:
                    cs = slice(c0, c0 + nk)
                    gk = slice(kh * KHW + c0, kh * KHW + c0 + nk)
                    wpat = bass.AP(tensor=wta.tensor, offset=wta.offset,
                                   ap=[list(wta.ap[0]), [0, nk], [1, E]])
                    nc.vector.tensor_mul(wef[:, cs, :], ef_t[:, cs, :], wpat)
                    nc.vector.tensor_add(wef[:, cs, 0:8], wef[:, cs, 0:8],
                                         wef[:, cs, 8:16])
                    nc.vector.tensor_add(wef[:, cs, 0:4], wef[:, cs, 0:4],
                                         wef[:, cs, 4:8])
                    nc.vector.tensor_add(P[:, cs, :], wef[:, cs, 0:2],
                                         wef[:, cs, 2:4])
                    nc.vector.scalar_tensor_tensor(
                        out=st["acc"][:, gk], in0=P[:, cs, 0], scalar=1.0,
                        in1=P[:, cs, 1], op0=ALU.mult, op1=ALU.add)

            def emit_gps_att2(j):
                # acc = (acc + att1[r]) + att2[k]  in one STT
                hj = halves[j]
                rt, kh = hj["rt"], hj["kh"]
                st = rts[rt]
                eng = nc.gpsimd if use_gps else nc.vector
                for (c0, nk) in hj["chunks"]:
                    gk = slice(kh * KHW + c0, kh * KHW + c0 + nk)
                    eng.scalar_tensor_tensor(
                        out=st["acc"][:, gk], in0=st["acc"][:, gk],
                        scalar=att1p[:, rt:rt + 1], in1=att2b[:, gk],
                        op0=ALU.add, op1=ALU.add)

            def emit_acts(j):
                hj = halves[j]
                rt, kh = hj["rt"], hj["kh"]
                st = rts[rt]
                hj["ex"] = work.tile([128, KHW], fp32, tag="ex", name=f"ex{j}")
                for (c0, nk) in hj["chunks"]:
                    cs = slice(c0, c0 + nk)
                    gk = slice(kh * KHW + c0, kh * KHW + c0 + nk)
                    if relu_mode:
                        # lrelu(z) = z + 0.99*relu(-z)
                        rn = work.tile([128, KHW], fp32, tag="rn")
                        nc.scalar.activation(rn[:, cs], st["acc"][:, gk],
                                             AF.Relu, scale=-1.0)
                        lkp = work.tile([128, KHW], fp32, tag="lk")
                        nc.vector.scalar_tensor_tensor(
                            out=lkp[:, cs], in0=rn[:, cs], scalar=0.99,
                            in1=st["acc"][:, gk], op0=ALU.mult, op1=ALU.add)
                        nc.scalar.activation(hj["ex"][:, cs], lkp[:, cs],
                                             AF.Exp)
                    else:
                        lk = work.tile([128, KHW], fp32, tag="lk")
                        nc.scalar.activation(lk[:, cs], st["acc"][:, gk],
                                             AF.Prelu, alpha=0.01)
                        nc.scalar.activation(hj["ex"][:, cs], lk[:, cs],
                                             AF.Exp)

            def emit_ttr(j):
                hj = halves[j]
                rt, kh = hj["rt"], hj["kh"]
                st = rts[rt]
                for (c0, nk) in hj["chunks"]:
                    cs = slice(c0, c0 + nk)
                    gk = slice(kh * KHW + c0, kh * KHW + c0 + nk)
                    s_new = small.tile([128, 1], fp32,
                                       tag=f"s{st['nseg'] % 4}",
                                       name=f"s_{rt}_{st['nseg']}")
                    if use_ttr:
                        seed = 0.0 if st["nseg"] == 0 else st["s"][-1][:]
                        nc.vector.tensor_tensor_reduce(
                            out=st["coefs"][:, gk], in0=hj["ex"][:, cs],
                            in1=st["adj"][:, gk], scale=1.0, scalar=seed,
                            op0=ALU.mult, op1=ALU.add, accum_out=s_new)
                    else:
                        nc.vector.scalar_tensor_tensor(
                            out=st["coefs"][:, gk], in0=hj["ex"][:, cs],
                            scalar=1.0, in1=st["adj"][:, gk],
                            op0=ALU.mult, op1=ALU.mult, accum_out=s_new)
                        if st["nseg"] > 0:
                            nc.vector.tensor_add(s_new, s_new,
                                                 st["s"][-1][:])
                    st["s"].append(s_new)
                    st["nseg"] += 1

            def emit_av(j):
                hj = halves[j]
                rt, kh = hj["rt"], hj["kh"]
                st = rts[rt]
                if "ret" not in st:
                    st["ret"] = psR.tile([128, O], fp32, tag="ret", name=f"ret{rt}")
                for kcl in range(4):
                    kc = kh * 4 + kcl
                    tp = psT.tile([128, 128], fp32, tag="tp")
                    nc.tensor.transpose(
                        tp, st["coefs"][:, kc * 128:(kc + 1) * 128], ident_sb)
                    ctT = small.tile([128, 128], bf16, tag="ctT")
                    nc.scalar.copy(out=ctT, in_=tp)
                    nc.tensor.matmul(st["ret"], ctT, V[:, kc, :],
                                     start=(kc == 0), stop=(kc == KC - 1))

            def emit_skip(rt):
                st = ensure_rt(rt)
                rsl = slice(rt * 128, (rt + 1) * 128)
                skp = psS.tile([128, O], fp32, tag="skp")
                nc.tensor.matmul(skp, fTr0[:, rsl], Wsk_sb[:, 0, :],
                                 start=True, stop=False)
                nc.tensor.matmul(skp, fTr1[:, rsl], Wsk_sb[:, 1, :],
                                 start=False, stop=False)
                nc.tensor.matmul(skp, ones_f, bsk_sb,
                                 start=False, stop=True)
                st["sk"] = small.tile([128, O], fp32, tag="sksb", name=f"sk{rt}")
                nc.scalar.copy(out=st["sk"], in_=skp)

            def emit_rtend(rt):
                st = rts[rt]
                r = small.tile([128, 1], fp32, tag="r")
                if use_recip:
                    nc.vector.reciprocal(r, st["s"][-1])
                else:
                    lns = small.tile([128, 1], fp32, tag="lns")
                    nc.scalar.activation(lns, st["s"][-1], AF.Ln)
                    nc.scalar.activation(r, lns, AF.Exp, scale=-1.0)
                out_sb = work.tile([128, O], fp32, tag="outsb")
                nc.vector.scalar_tensor_tensor(
                    out=out_sb, in0=st["ret"], scalar=r, in1=st["sk"],
                    op0=ALU.mult, op1=ALU.add)
                rsl = slice(rt * 128, (rt + 1) * 128)
                nc.sync.dma_start(out=out_t[rsl, :], in_=out_sb)

            # --- pipeline: iteration j emits ---------------------------
            #   sync   : adj prefetch (rt of half j+2)
            #   gpsimd : ef triggers for half j+1, then att2add(j-1)
            #   PE/Sc  : skip(rt) when half j opens a row tile
            #   DVE    : dve_a(j), ttr(j-1), rtend(rt closed at j-2)
            #   Scalar : acts(j-1)
            #   PE/Sc  : AV(j-2)
            emit_adj(0)
            emit_adj(1)
            emit_trig(0)
            emit_trig(1)
            for j in range(J + 2):
                hj = halves[j] if j < J else None
                if hj is not None:
                    if hj["kh"] == 0 and hj["rt"] + 2 < RT:
                        emit_adj(hj["rt"] + 2)
                    if j + 1 < J:
                        emit_trig(j + 1)
                    if hj["kh"] == 0:
                        emit_skip(hj["rt"])
                    emit_dve_a(j)
                if j >= 1 and j - 1 < J:
                    emit_gps_att2(j - 1)
                    emit_acts(j - 1)
                    emit_ttr(j - 1)
                if j >= 2 and j - 2 < J:
                    emit_av(j - 2)
                    if halves[j - 2]["last"]:
                        emit_rtend(halves[j - 2]["rt"])

    nc.compile()
    return nc


def _get_nc():
    if "nc" not in _cache:
        _cache["nc"] = _build()
    return _cache["nc"]


def _in_maps(hidden, n_features, e_features, g_features, adj,
             W_m, b_m, W_skip, b_skip, w_a1, b_a1, w_a2, b_a2,
             w_ae, b_ae, w_ag, b_ag):
    f32 = np.float32
    asf = lambda x: np.ascontiguousarray(np.asarray(x, dtype=f32))
    shared = {
        "Wm": asf(W_m), "bm": asf(b_m).reshape(1, O),
        "Wsk": asf(W_skip), "bsk": asf(b_skip).reshape(1, O),
        "wa1": asf(w_a1), "wa2": asf(w_a2),
        "wae": asf(w_ae).reshape(1, E), "wag": asf(w_ag),
        "bs14": np.array([[np.float32(np.asarray(b_a1).reshape(())),
                          np.float32(np.asarray(b_a2).reshape(())),
                          np.float32(np.asarray(b_ae).reshape(())),
                          np.float32(np.asarray(b_ag).reshape(()))]],
                         dtype=f32),
        "bs4": np.array([[np.float32(np.asarray(b_a1).reshape(()))],
                         [np.float32(np.asarray(b_a2).reshape(()))],
                         [np.float32(np.asarray(b_ae).reshape(()))],
                         [np.float32(np.asarray(b_ag).reshape(()))]],
                        dtype=f32),
        "ident": np.eye(128, dtype=f32),
    }
    maps = []
    for c in range(NCORES):
        b, h = c // 2, c % 2
        rows = slice(h * ROWS, (h + 1) * ROWS)
        m = dict(shared)
        m["ef"] = asf(e_features[b, rows])
        m["adj"] = asf(adj[b, rows])
        m["nfk"] = asf(n_features[b])
        m["hidk"] = asf(hidden[b])
        m["nfr"] = asf(n_features[b][rows])
        m["hidr"] = asf(hidden[b][rows])
        m["g"] = asf(g_features[b]).reshape(G, 1)
        maps.append(m)
    return maps


def kernel(hidden, n_features, e_features, g_features, adj,
           W_m, b_m, W_skip, b_skip, w_a1, b_a1, w_a2, b_a2,
           w_ae, b_ae, w_ag, b_ag):
    from concourse import bass_utils
    nc = _get_nc()
    maps = _in_maps(hidden, n_features, e_features, g_features, adj,
                    W_m, b_m, W_skip, b_skip, w_a1, b_a1, w_a2, b_a2,
                    w_ae, b_ae, w_ag, b_ag)
    res = bass_utils.run_bass_kernel_spmd(nc, maps, core_ids=list(range(NCORES)))
    out = np.empty((B, N, O), np.float32)
    for c in range(NCORES):
        b, h = c // 2, c % 2
        out[b, h * ROWS:(h + 1) * ROWS] = res.results[c]["out"]
    return out


# revision 11
# speedup vs baseline: 1.2574x; 1.2574x over previous
"""GAT message-passing kernel for Trainium2, 8 NeuronCores — v2.

Problem (hardcoded): B=4, N=1024, H=F=O=G=128, E=16.
  features = concat([n_features, hidden], -1)            [B,N,256]
  values   = features @ W_m + b_m                        [B,N,128]
  logits   = att1 + att2^T + (e_features@w_ae) + att_g   [B,N,N]
  coefs    = softmax(leaky_relu(logits) + (adj-1)*1e9)
  out      = coefs @ values + features @ W_skip + b_skip

Sharding: 8 cores = (batch b = core//2) x (row half = core%2).
Each core handles 512 query rows of one batch. No collectives.

v2 design (from trace analysis of v1 @156us; DMA stream floor ~101us):
  - DVE instruction stream contains ONLY the steady-state work: per
    512-key half  mul(2x) -> tree L1/L2(2x) -> L3 into contiguous pair
    buffer -> STT final (no stride-16 singles)  ~9.6us < 11.9us DMA
    period.  v1 spent 13.3us/half (strided tree tail) and its DVE
    stream was blocked 33us behind phase-0.
  - att1+att_g+biases ride the ScalarE activation bias (per-partition);
    att2 is broadcast across partitions once via PE and added by
    GpSimd per half, so logit assembly costs DVE nothing.
  - Prelu (parametric relu, alpha=.01) shares the activation table
    with Exp/Copy -> zero ACT_TABLE_LOADs (v1 burned 21.8us on 17).
  - softmax denominator via TTR accum chaining per chunk; 1/s via
    vector.reciprocal (kills the Ln/Exp table thrash).
  - software-pipelined emission: acts lag one half, A@V lags two, ef
    DMA triggers always one period ahead on the gpsimd queue.
  - first and last halves stream in 256-key quarters to cut the
    pipeline head (first mul ~11.5us) and tail (~9us after last byte).
"""

import os
import numpy as np

B, N, H, F, E, G, O = 4, 1024, 128, 128, 16, 128, 128
DIN = F + H
NCORES = 8
ROWS = N // 2          # query rows per core
RT = ROWS // 128       # row tiles per core
KC = N // 128          # key chunks per row
KH = 2                 # key halves per row
KHW = N // KH          # keys per half

_cache = {}


def _build(stage=4):
    from contextlib import ExitStack
    import concourse.bacc as bacc
    import concourse.tile as tile
    import concourse.mybir as mybir
    import concourse.bass as bass

    fp32 = mybir.dt.float32
    bf16 = mybir.dt.bfloat16
    ALU = mybir.AluOpType
    AF = mybir.ActivationFunctionType
    relu_mode = bool(os.environ.get("GAT_LRELU_MODE"))  # Relu+STT fallback
    use_gps = os.environ.get("GAT_GPS", "0") == "1"    # gpsimd compute ops
    use_ttr = os.environ.get("GAT_TTR", "0") == "1"    # tensor_tensor_reduce
    use_recip = os.environ.get("GAT_RECIP", "1") == "1"  # vector.reciprocal
    use_p4 = os.environ.get("GAT_P4", "0") == "1"      # P=4 bias matmul

    nc = bacc.Bacc("TRN2", target_bir_lowering=False, debug=False,
                   num_devices=NCORES)

    # ---- per-core I/O -------------------------------------------------
    ef_in = nc.dram_tensor("ef", [ROWS, N, E], fp32, kind="ExternalInput")
    adj_in = nc.dram_tensor("adj", [ROWS, N], fp32, kind="ExternalInput")
    nfk_in = nc.dram_tensor("nfk", [N, F], fp32, kind="ExternalInput")
    hidk_in = nc.dram_tensor("hidk", [N, H], fp32, kind="ExternalInput")
    g_in = nc.dram_tensor("g", [G, 1], fp32, kind="ExternalInput")
    Wm_in = nc.dram_tensor("Wm", [DIN, O], fp32, kind="ExternalInput")
    bm_in = nc.dram_tensor("bm", [1, O], fp32, kind="ExternalInput")
    Wsk_in = nc.dram_tensor("Wsk", [DIN, O], fp32, kind="ExternalInput")
    bsk_in = nc.dram_tensor("bsk", [1, O], fp32, kind="ExternalInput")
    wa1_in = nc.dram_tensor("wa1", [DIN, 1], fp32, kind="ExternalInput")
    wa2_in = nc.dram_tensor("wa2", [DIN, 1], fp32, kind="ExternalInput")
    wae_in = nc.dram_tensor("wae", [1, E], fp32, kind="ExternalInput")
    wag_in = nc.dram_tensor("wag", [G, 1], fp32, kind="ExternalInput")
    bs4_in = nc.dram_tensor("bs4", [4, 1], fp32, kind="ExternalInput")
    bs14_in = nc.dram_tensor("bs14", [1, 4], fp32, kind="ExternalInput")
    ident_in = nc.dram_tensor("ident", [128, 128], fp32, kind="ExternalInput")
    out_t = nc.dram_tensor("out", [ROWS, O], fp32, kind="ExternalOutput")

    with tile.TileContext(nc) as tc:
        with ExitStack() as ctx:
            singles = ctx.enter_context(tc.tile_pool(name="singles", bufs=1))
            efp = ctx.enter_context(tc.tile_pool(name="efp", bufs=4))
            wefp = ctx.enter_context(tc.tile_pool(name="wefp", bufs=2))
            pairp = ctx.enter_context(tc.tile_pool(name="pairp", bufs=2))
            work = ctx.enter_context(tc.tile_pool(name="work", bufs=2))
            adjp = ctx.enter_context(tc.tile_pool(name="adjp", bufs=3))
            small = ctx.enter_context(tc.tile_pool(name="small", bufs=2))
            psT = ctx.enter_context(tc.tile_pool(name="psT", bufs=3, space="PSUM"))
            psR = ctx.enter_context(tc.tile_pool(name="psR", bufs=2, space="PSUM"))
            psS = ctx.enter_context(tc.tile_pool(name="psS", bufs=2, space="PSUM"))

            # ============ GpSimd head: memsets + wae bcast ==============
            eng0 = nc.gpsimd if use_gps else nc.vector
            ones_bf = singles.tile([1, 512], bf16)
            eng0.memset(ones_bf, 1.0)
            ones128b = ones_bf[:, :128]
            ones_f = singles.tile([1, 128], fp32)
            eng0.memset(ones_f, 1.0)
            ones4 = singles.tile([4, 1], fp32)
            eng0.memset(ones4, 1.0)
            w_tile = singles.tile([128, E], bf16)       # w_ae bcast to parts
            nc.gpsimd.dma_start(out=w_tile, in_=bass.AP(
                tensor=wae_in, offset=0, ap=[[0, 128], [1, E]]))

            # ============ input loads: all on Q0 (gpsimd), priority =====
            # order = transfer order; everything phase-0 lands before the
            # ef stream saturates the engines.  sync queue only writes.
            ident_sb = singles.tile([128, 128], fp32)
            nc.gpsimd.dma_start(out=ident_sb, in_=ident_in.ap())
            g_sb = singles.tile([128, 1], fp32)
            nc.gpsimd.dma_start(out=g_sb, in_=g_in.ap())
            wag_sb = singles.tile([128, 1], fp32)
            nc.gpsimd.dma_start(out=wag_sb, in_=wag_in.ap())
            bs4_sb = singles.tile([4, 1], fp32)
            nc.gpsimd.dma_start(out=bs4_sb, in_=bs4_in.ap())
            bs14_sb = singles.tile([1, 4], fp32)
            nc.gpsimd.dma_start(out=bs14_sb, in_=bs14_in.ap())
            wa1_sb = singles.tile([128, 2, 1], fp32)
            nc.gpsimd.dma_start(out=wa1_sb, in_=wa1_in.ap().rearrange(
                "(c p) o -> p c o", p=128))
            wa2_sb = singles.tile([128, 2, 1], bf16)
            nc.gpsimd.dma_start(out=wa2_sb, in_=wa2_in.ap().rearrange(
                "(c p) o -> p c o", p=128))

            # persistent outputs of phase 0
            fTk0 = singles.tile([128, N], bf16)    # n_features^T (keys)
            fTk1 = singles.tile([128, N], bf16)    # hidden^T (keys)
            fTr0 = singles.tile([128, ROWS], fp32)  # fp32 copies for rows
            fTr1 = singles.tile([128, ROWS], fp32)
            V = singles.tile([128, KC, O], bf16)
            Wm_sb = singles.tile([128, 2, O], bf16)
            nc.gpsimd.dma_start(out=Wm_sb, in_=Wm_in.ap().rearrange(
                "(c p) o -> p c o", p=128))
            bm_sb = singles.tile([1, O], bf16)
            nc.gpsimd.dma_start(out=bm_sb, in_=bm_in.ap())
            att2b = singles.tile([128, N], fp32)   # att2 bcast to all parts
            att1p = singles.tile([128, RT], fp32)  # att1 + att_g + biases
            sc_sb = singles.tile([1, 1], fp32)

            nfr_in = nc.dram_tensor("nfr", [ROWS, F], fp32,
                                    kind="ExternalInput")
            hidr_in = nc.dram_tensor("hidr", [ROWS, H], fp32,
                                     kind="ExternalInput")

            with tc.tile_pool(name="ph0", bufs=1) as ph0:
                nfr_sb = ph0.tile([128, RT, F], fp32)
                nc.gpsimd.dma_start(out=nfr_sb, in_=nfr_in.ap().rearrange(
                    "(c p) f -> p c f", p=128))
                hidr_sb = ph0.tile([128, RT, H], fp32)
                nc.gpsimd.dma_start(out=hidr_sb, in_=hidr_in.ap().rearrange(
                    "(c p) f -> p c f", p=128))
                nfk_sb = ph0.tile([128, KC, F], fp32)
                nc.gpsimd.dma_start(out=nfk_sb, in_=nfk_in.ap().rearrange(
                    "(c p) f -> p c f", p=128))
                hidk_sb = ph0.tile([128, KC, H], fp32)
                nc.gpsimd.dma_start(out=hidk_sb, in_=hidk_in.ap().rearrange(
                    "(c p) f -> p c f", p=128))
                Wsk_sb = singles.tile([128, 2, O], fp32)
                nc.gpsimd.dma_start(out=Wsk_sb, in_=Wsk_in.ap().rearrange(
                    "(c p) o -> p c o", p=128))
                bsk_sb = singles.tile([1, O], fp32)
                nc.gpsimd.dma_start(out=bsk_sb, in_=bsk_in.ap())

                # sc = g@w_ag + (b_a1+b_a2+b_ae+b_ag)   [1,1]
                scps = psR.tile([1, 1], fp32, tag="ret")
                nc.tensor.matmul(scps, g_sb, wag_sb, start=True,
                                 stop=not use_p4)
                if use_p4:
                    nc.tensor.matmul(scps, bs4_sb, ones4,
                                     start=False, stop=True)
                    nc.scalar.copy(out=sc_sb, in_=scps)
                else:
                    nc.scalar.copy(out=sc_sb, in_=scps)
                    for i in range(4):
                        nc.vector.tensor_scalar_add(sc_sb, sc_sb,
                                                    bs14_sb[:, i:i + 1])

                # --- att1 path first (feeds the first bias-fold STT) ----
                for rc in range(RT):
                    rsl = slice(rc * 128, (rc + 1) * 128)
                    tp = psT.tile([128, 128], fp32, tag="tp")
                    nc.tensor.transpose(tp, nfr_sb[:, rc, :], ident_sb)
                    nc.scalar.copy(out=fTr0[:, rsl], in_=tp)
                    tp2 = psT.tile([128, 128], fp32, tag="tp")
                    nc.tensor.transpose(tp2, hidr_sb[:, rc, :], ident_sb)
                    nc.scalar.copy(out=fTr1[:, rsl], in_=tp2)
                for rc in range(RT):
                    rsl = slice(rc * 128, (rc + 1) * 128)
                    aps = psR.tile([128, 1], fp32, tag="ret")
                    nc.tensor.matmul(aps, fTr0[:, rsl], wa1_sb[:, 0, :],
                                     start=True, stop=False)
                    nc.tensor.matmul(aps, fTr1[:, rsl], wa1_sb[:, 1, :],
                                     start=False, stop=False)
                    nc.tensor.matmul(aps, ones_f, sc_sb,
                                     start=False, stop=True)
                    nc.scalar.copy(out=att1p[:, rc:rc + 1], in_=aps)

                # --- keys^T, att2 + broadcast, then V -------------------
                for kc in range(KC):
                    ksl = slice(kc * 128, (kc + 1) * 128)
                    tp = psT.tile([128, 128], fp32, tag="tp")
                    nc.tensor.transpose(tp, nfk_sb[:, kc, :], ident_sb)
                    nc.scalar.copy(out=fTk0[:, ksl], in_=tp)
                    tp2 = psT.tile([128, 128], fp32, tag="tp")
                    nc.tensor.transpose(tp2, hidk_sb[:, kc, :], ident_sb)
                    nc.scalar.copy(out=fTk1[:, ksl], in_=tp2)

                att2_sb = ph0.tile([1, N], fp32)
                for khf in range(2):
                    ksl = slice(khf * 512, (khf + 1) * 512)
                    a2ps = psR.tile([1, 512], fp32, tag="ret")
                    nc.tensor.matmul(a2ps, wa2_sb[:, 0, :], fTk0[:, ksl],
                                     start=True, stop=False)
                    nc.tensor.matmul(a2ps, wa2_sb[:, 1, :], fTk1[:, ksl],
                                     start=False, stop=True)
                    nc.scalar.copy(out=att2_sb[:, ksl], in_=a2ps)
                for khf in range(2):
                    ksl = slice(khf * 512, (khf + 1) * 512)
                    bcps = psR.tile([128, 512], fp32, tag="ret")
                    nc.tensor.matmul(bcps, ones_f, att2_sb[:, ksl],
                                     start=True, stop=True)
                    nc.scalar.copy(out=att2b[:, ksl], in_=bcps)

                for kc in range(KC):
                    vps = psR.tile([128, O], fp32, tag="ret")
                    ksl = slice(kc * 128, (kc + 1) * 128)
                    nc.tensor.matmul(vps, fTk0[:, ksl], Wm_sb[:, 0, :],
                                     start=True, stop=False)
                    nc.tensor.matmul(vps, fTk1[:, ksl], Wm_sb[:, 1, :],
                                     start=False, stop=False)
                    nc.tensor.matmul(vps, ones128b, bm_sb,
                                     start=False, stop=True)
                    nc.scalar.copy(out=V[:, kc, :], in_=vps)

            # ============ streaming pipeline ============================
            # halves: j = rt*2 + kh; first and last halves stream as two
            # 256-key quarters, the rest as one 512-key chunk.
            halves = []
            for rt in range(RT):
                for kh in range(2):
                    j = rt * 2 + kh
                    quartered = (j == 0) or (j == RT * 2 - 1)
                    chunks = [(0, 256), (256, 256)] if quartered \
                        else [(0, 512)]
                    halves.append(dict(rt=rt, kh=kh, chunks=chunks,
                                       last=(kh == 1)))
            J = len(halves)

            wta = w_tile[:]

            rts = [dict() for _ in range(RT)]   # per-row-tile state

            def ensure_rt(rt):
                st = rts[rt]
                if "acc" in st:
                    return st
                st["acc"] = work.tile([128, N], fp32, tag="acc", name=f"acc{rt}")
                st["coefs"] = work.tile([128, N], fp32, tag="coefs", name=f"coefs{rt}")
                st["s"] = []
                st["nseg"] = 0
                return st

            def emit_adj(rt):
                st = ensure_rt(rt)
                st["adj"] = adjp.tile([128, N], fp32, tag="adj", name=f"adj{rt}")
                rsl = slice(rt * 128, (rt + 1) * 128)
                nc.gpsimd.dma_start(out=st["adj"], in_=adj_in[rsl, :])

            def emit_trig(j):
                hj = halves[j]
                rt, kh = hj["rt"], hj["kh"]
                rsl = slice(rt * 128, (rt + 1) * 128)
                hj["ef"] = efp.tile([128, KHW, E], bf16, tag="ef", name=f"ef{j}")
                for (c0, nk) in hj["chunks"]:
                    gk = slice(kh * KHW + c0, kh * KHW + c0 + nk)
                    nc.gpsimd.dma_start(out=hj["ef"][:, c0:c0 + nk, :],
                                        in_=ef_in[rsl, gk, :])

            def emit_dve_a(j):
                hj = halves[j]
                rt, kh = hj["rt"], hj["kh"]
                st = ensure_rt(rt)
                hj["wef"] = wefp.tile([128, KHW, E], bf16, tag="wef", name=f"wef{j}")
                hj["P"] = pairp.tile([128, KHW, 2], bf16, tag="P", name=f"P{j}")
                ef_t, wef, P = hj["ef"], hj["wef"], hj["P"]
                for (c0, nk) in hj["chunks"]:
                    cs = slice(c0, c0 + nk)
                    gk = slice(kh * KHW + c0, kh * KHW + c0 + nk)
                    wpat = bass.AP(tensor=wta.tensor, offset=wta.offset,
                                   ap=[list(wta.ap[0]), [0, nk], [1, E]])
                    nc.vector.tensor_mul(wef[:, cs, :], ef_t[:, cs, :], wpat)
                    nc.vector.tensor_add(wef[:, cs, 0:8], wef[:, cs, 0:8],
                                         wef[:, cs, 8:16])
                    nc.vector.tensor_add(wef[:, cs, 0:4], wef[:, cs, 0:4],
                                         wef[:, cs, 4:8])
                    nc.vector.tensor_add(P[:, cs, :], wef[:, cs, 0:2],
                                         wef[:, cs, 2:4])
                    nc.vector.scalar_tensor_tensor(
                        out=st["acc"][:, gk], in0=P[:, cs, 0], scalar=1.0,
                        in1=P[:, cs, 1], op0=ALU.mult, op1=ALU.add)

            def emit_gps_att2(j):
                # acc = (acc + att1[r]) + att2[k]  in one STT
                hj = halves[j]
                rt, kh = hj["rt"], hj["kh"]
                st = rts[rt]
                eng = nc.gpsimd if use_gps else nc.vector
                for (c0, nk) in hj["chunks"]:
                    gk = slice(kh * KHW + c0, kh * KHW + c0 + nk)
                    eng.scalar_tensor_tensor(
                        out=st["acc"][:, gk], in0=st["acc"][:, gk],
                        scalar=att1p[:, rt:rt + 1], in1=att2b[:, gk],
                        op0=ALU.add, op1=ALU.add)

            def emit_acts(j):
                hj = halves[j]
                rt, kh = hj["rt"], hj["kh"]
                st = rts[rt]
                hj["ex"] = work.tile([128, KHW], fp32, tag="ex", name=f"ex{j}")
                for (c0, nk) in hj["chunks"]:
                    cs = slice(c0, c0 + nk)
                    gk = slice(kh * KHW + c0, kh * KHW + c0 + nk)
                    if relu_mode:
                        # lrelu(z) = z + 0.99*relu(-z)
                        rn = work.tile([128, KHW], fp32, tag="rn")
                        nc.scalar.activation(rn[:, cs], st["acc"][:, gk],
                                             AF.Relu, scale=-1.0)
                        lkp = work.tile([128, KHW], fp32, tag="lk")
                        nc.vector.scalar_tensor_tensor(
                            out=lkp[:, cs], in0=rn[:, cs], scalar=0.99,
                            in1=st["acc"][:, gk], op0=ALU.mult, op1=ALU.add)
                        nc.scalar.activation(hj["ex"][:, cs], lkp[:, cs],
                                             AF.Exp)
                    else:
                        lk = work.tile([128, KHW], fp32, tag="lk")
                        nc.scalar.activation(lk[:, cs], st["acc"][:, gk],
                                             AF.Prelu, alpha=0.01)
                        nc.scalar.activation(hj["ex"][:, cs], lk[:, cs],
                                             AF.Exp)

            def emit_ttr(j):
                hj = halves[j]
                rt, kh = hj["rt"], hj["kh"]
                st = rts[rt]
                for (c0, nk) in hj["chunks"]:
                    cs = slice(c0, c0 + nk)
                    gk = slice(kh * KHW + c0, kh * KHW + c0 + nk)
                    s_new = small.tile([128, 1], fp32,
                                       tag=f"s{st['nseg'] % 4}",
                                       name=f"s_{rt}_{st['nseg']}")
                    if use_ttr:
                        seed = 0.0 if st["nseg"] == 0 else st["s"][-1][:]
                        nc.vector.tensor_tensor_reduce(
                            out=st["coefs"][:, gk], in0=hj["ex"][:, cs],
                            in1=st["adj"][:, gk], scale=1.0, scalar=seed,
                            op0=ALU.mult, op1=ALU.add, accum_out=s_new)
                    else:
                        nc.vector.scalar_tensor_tensor(
                            out=st["coefs"][:, gk], in0=hj["ex"][:, cs],
                            scalar=1.0, in1=st["adj"][:, gk],
                            op0=ALU.mult, op1=ALU.mult, accum_out=s_new)
                        if st["nseg"] > 0:
                            nc.vector.tensor_add(s_new, s_new,
                                                 st["s"][-1][:])
                    st["s"].append(s_new)
                    st["nseg"] += 1

            def emit_av(j):
                hj = halves[j]
                rt, kh = hj["rt"], hj["kh"]
                st = rts[rt]
                if "ret" not in st:
                    st["ret"] = psR.tile([128, O], fp32, tag="ret", name=f"ret{rt}")
                for kcl in range(4):
                    kc = kh * 4 + kcl
                    tp = psT.tile([128, 128], fp32, tag="tp")
                    nc.tensor.transpose(
                        tp, st["coefs"][:, kc * 128:(kc + 1) * 128], ident_sb)
                    ctT = small.tile([128, 128], bf16, tag="ctT")
                    nc.scalar.copy(out=ctT, in_=tp)
                    nc.tensor.matmul(st["ret"], ctT, V[:, kc, :],
                                     start=(kc == 0), stop=(kc == KC - 1))

            def emit_skip(rt):
                st = ensure_rt(rt)
                rsl = slice(rt * 128, (rt + 1) * 128)
                skp = psS.tile([128, O], fp32, tag="skp")
                nc.tensor.matmul(skp, fTr0[:, rsl], Wsk_sb[:, 0, :],
                                 start=True, stop=False)
                nc.tensor.matmul(skp, fTr1[:, rsl], Wsk_sb[:, 1, :],
                                 start=False, stop=False)
                nc.tensor.matmul(skp, ones_f, bsk_sb,
                                 start=False, stop=True)
                st["sk"] = small.tile([128, O], fp32, tag="sksb", name=f"sk{rt}")
                nc.scalar.copy(out=st["sk"], in_=skp)

            def emit_rtend(rt):
                st = rts[rt]
                r = small.tile([128, 1], fp32, tag="r")
                if use_recip:
                    nc.vector.reciprocal(r, st["s"][-1])
                else:
                    lns = small.tile([128, 1], fp32, tag="lns")
                    nc.scalar.activation(lns, st["s"][-1], AF.Ln)
                    nc.scalar.activation(r, lns, AF.Exp, scale=-1.0)
                out_sb = work.tile([128, O], fp32, tag="outsb")
                nc.vector.scalar_tensor_tensor(
                    out=out_sb, in0=st["ret"], scalar=r, in1=st["sk"],
                    op0=ALU.mult, op1=ALU.add)
                rsl = slice(rt * 128, (rt + 1) * 128)
                nc.sync.dma_start(out=out_t[rsl, :], in_=out_sb)

            # --- pipeline: iteration j emits ---------------------------
            #   sync   : adj prefetch (rt of half j+2)
            #   gpsimd : ef triggers for half j+1, then att2add(j-1)
            #   PE/Sc  : skip(rt) when half j opens a row tile
            #   DVE    : dve_a(j), ttr(j-1), rtend(rt closed at j-2)
            #   Scalar : acts(j-1)
            #   PE/Sc  : AV(j-2)
            emit_adj(0)
            emit_trig(0)
            emit_trig(1)
            emit_adj(1)
            for j in range(J + 2):
                hj = halves[j] if j < J else None
                if hj is not None:
                    if hj["kh"] == 0 and hj["rt"] + 2 < RT:
                        emit_adj(hj["rt"] + 2)
                    if j + 1 < J:
                        emit_trig(j + 1)
                    if hj["kh"] == 0:
                        emit_skip(hj["rt"])
                    emit_dve_a(j)
                if j >= 1 and j - 1 < J:
                    emit_gps_att2(j - 1)
                    emit_acts(j - 1)
                    emit_ttr(j - 1)
                if j >= 2 and j - 2 < J:
                    emit_av(j - 2)
                    if halves[j - 2]["last"]:
                        emit_rtend(halves[j - 2]["rt"])

    nc.compile()
    return nc


def _get_nc():
    if "nc" not in _cache:
        _cache["nc"] = _build()
    return _cache["nc"]


def _in_maps(hidden, n_features, e_features, g_features, adj,
             W_m, b_m, W_skip, b_skip, w_a1, b_a1, w_a2, b_a2,
             w_ae, b_ae, w_ag, b_ag):
    f32 = np.float32
    asf = lambda x: np.ascontiguousarray(np.asarray(x, dtype=f32))
    shared = {
        "Wm": asf(W_m), "bm": asf(b_m).reshape(1, O),
        "Wsk": asf(W_skip), "bsk": asf(b_skip).reshape(1, O),
        "wa1": asf(w_a1), "wa2": asf(w_a2),
        "wae": asf(w_ae).reshape(1, E), "wag": asf(w_ag),
        "bs14": np.array([[np.float32(np.asarray(b_a1).reshape(())),
                          np.float32(np.asarray(b_a2).reshape(())),
                          np.float32(np.asarray(b_ae).reshape(())),
                          np.float32(np.asarray(b_ag).reshape(()))]],
                         dtype=f32),
        "bs4": np.array([[np.float32(np.asarray(b_a1).reshape(()))],
                         [np.float32(np.asarray(b_a2).reshape(()))],
                         [np.float32(np.asarray(b_ae).reshape(()))],
                         [np.float32(np.asarray(b_ag).reshape(()))]],
                        dtype=f32),
        "ident": np.eye(128, dtype=f32),
    }
    maps = []
    for c in range(NCORES):
        b, h = c // 2, c % 2
        rows = slice(h * ROWS, (h + 1) * ROWS)
        m = dict(shared)
        m["ef"] = asf(e_features[b, rows])
        m["adj"] = asf(adj[b, rows])
        m["nfk"] = asf(n_features[b])
        m["hidk"] = asf(hidden[b])
        m["nfr"] = asf(n_features[b][rows])
        m["hidr"] = asf(hidden[b][rows])
        m["g"] = asf(g_features[b]).reshape(G, 1)
        maps.append(m)
    return maps


def kernel(hidden, n_features, e_features, g_features, adj,
           W_m, b_m, W_skip, b_skip, w_a1, b_a1, w_a2, b_a2,
           w_ae, b_ae, w_ag, b_ag):
    from concourse import bass_utils
    nc = _get_nc()
    maps = _in_maps(hidden, n_features, e_features, g_features, adj,
                    W_m, b_m, W_skip, b_skip, w_a1, b_a1, w_a2, b_a2,
                    w_ae, b_ae, w_ag, b_ag)
    res = bass_utils.run_bass_kernel_spmd(nc, maps, core_ids=list(range(NCORES)))
    out = np.empty((B, N, O), np.float32)
    for c in range(NCORES):
        b, h = c // 2, c % 2
        out[b, h * ROWS:(h + 1) * ROWS] = res.results[c]["out"]
    return out
